# revision 10
# baseline (speedup 1.0000x reference)
"""LocallyConnected1d (untied-weight conv1d) on 8 Trainium2 NeuronCores.

Problem (hardcoded):
    x:      (B=128, C=64, L=1024) f32
    weight: (O=64, C=64, P=1024, K=7) f32   (untied per output position)
    bias:   (O=64, P=1024) f32
    out:    (B=128, O=64, P=1024) f32
    out[b,o,p] = sum_{c,k} xpad[b,c,p+k] * w[o,c,p,k] + bias[o,p]  (pad=3)

Sharding: sequence-parallel over P — core m owns positions [128m, 128m+128).
Each weight element is used exactly once, so this minimizes HBM traffic.

The kernel is HBM-DMA-bound, so everything streams as fp16 (x, weight,
bias, output): per-core traffic is ~12.2 MB vs 23.4 MB for f32, and fp16
matmul runs at full PE rate. Accumulation stays in f32 PSUM; fp16
quantization of inputs+output keeps rel-err ~1e-3.

Per-core algorithm: adjacent input columns (2t, 2t+1) are stacked into
128 contraction rows (2 cols x 64 ch), so each matmul uses the full PE
height. The stationary operand is the x column-pair [(h,c)=128, b=128];
the moving operand is that pair's untied weights [(h,c)=128, n*64] for
the n output positions the pair feeds inside the current PSUM bank of 8
positions (bank = [b=128, (pr,o)=512] f32 = one 2KB bank). A bank needs
column pairs 4g..4g+6.

A pair covering a position at the edge of its k-window only uses one of
its two columns; the other column's 64 rows would be zeros. For the
four k=7-side edges per bank (blocks i=3..6, even bank positions
0/2/4/6, always the h=0 column with k=6) the zero half is not stored:
those positions get a dense 64-row base-0 matmul from a separate strip
pack instead. The k=-1-side edges (h=1-only, which would need base-64
matmuls that cannot share a PSUM bank with base-0 ones) stay zero-padded
in the main pack. Bias opens each bank's accumulation group via a K=1
ones-x-bias matmul; eviction downcasts PSUM f32 -> fp16 on DVE/ACT.
"""

import numpy as np

B = 128
C = 64
O = 64
L = 1024
KW = 7
PAD = 3
NCORES = 8
PC = L // NCORES          # positions per core = 128
NJ = PC + 2 * PAD         # input columns per core incl halo = 134
NT = NJ // 2              # column pairs = 67
NBANK = PC // 8           # psum banks of 8 positions = 16
BANKW = 8 * O             # psum bank free width = 512 f32
XSPLIT = 36               # x pairs in the first DMA chunk (banks 0-7 use <=34)

# pair i = 4g+i covers bank-g positions [A0[i], A0[i]+N_I[i]) (bank-local)
N_I = [2, 4, 6, 8, 6, 4, 2]
A0 = [0, 0, 0, 0, 2, 4, 6]
# main-pack blocks: for i>=3 the first position (h=0-only, k=6) moves to the
# strip pack, so the stored block covers positions [A0[i]+S_I[i], ...).
S_I = [0, 0, 0, 1, 1, 1, 1]
WN = [N_I[i] - S_I[i] for i in range(7)]          # [2,4,6,7,5,3,1]
WOFF = np.cumsum([0] + WN).tolist()               # [0,2,6,12,19,24,27,28]
BANKC = WOFF[-1] * O                              # main cols per bank = 1792
WCOLS = NBANK * BANKC                             # main cols per core = 28672
SBANKC = 4 * O                                    # strip cols per bank = 256
SCOLS = NBANK * SBANKC                            # strip cols per core = 4096

# weight DMA chunks, as (first bank, first block, n blocks) — banks 0..13 in
# 2-bank chunks, then bank 14, then bank 15 split in two so the final
# compute chases the last bytes of the stream. Kept >=10 chunks of ~1MB so
# the ~8-deep HWDGE in-flight window always holds several MB of stream.
WCHUNKS = [(2 * q, 0, 14) for q in range(7)] + \
          [(14, 0, 7), (15, 0, 4), (15, 4, 3)]


def _chunk_cols(ch):
    g0, i0, nb = ch
    c0 = g0 * BANKC + WOFF[i0] * O
    i1 = i0 + nb
    c1 = (g0 + (i1 // 7)) * BANKC + WOFF[i1 % 7] * O
    return c0, c1


def _pack_inputs(x, weight, bias):
    """Host-side relayout into DMA/matmul-friendly fp16 per-core arrays."""
    f16 = np.float16
    xh = np.asarray(x, np.float32).astype(f16)
    wh = np.asarray(weight, np.float32).astype(f16)
    bh = np.asarray(bias, np.float32).astype(f16)

    xp = np.zeros((B, C, L + 2 * PAD), f16)
    xp[:, :, PAD:PAD + L] = xh
    xt = np.ascontiguousarray(xp.transpose(1, 2, 0))      # (C, 1030, B)
    xpacks = []
    for m in range(NCORES):
        s = xt[:, PC * m: PC * m + NJ, :]                 # (C, NJ, B)
        s = s.reshape(C, NT, 2, B).transpose(2, 0, 1, 3)  # (h, C, NT, B)
        xpacks.append(np.ascontiguousarray(s.reshape(2 * C, NT, B)))

    # main weight pack: W[m, h*C+c, g*BANKC + (WOFF[i]+pp')*64 + o]
    #   = w[o, c, 128m + 8g + A0[i] + S_I[i] + pp', k],
    #     k = 2i + h - A0[i] - S_I[i] - pp'   (zero where k outside [0,7))
    W = np.zeros((NCORES, 2, C, NBANK, WOFF[-1], O), f16)
    m_ = 128 * np.arange(NCORES)[:, None, None]
    g_ = 8 * np.arange(NBANK)[None, :, None]
    for i in range(7):
        for h in range(2):
            pp = np.arange(WN[i])
            kk = 2 * i + h - A0[i] - S_I[i] - pp
            sel = (kk >= 0) & (kk < KW)
            ppv, kkv = pp[sel], kk[sel]
            if ppv.size == 0:
                continue
            pg = m_ + g_ + A0[i] + S_I[i] + ppv[None, None, :]  # (M, G, nv)
            blk = wh[:, :, pg, kkv[None, None, :]]              # (O, C, M, G, nv)
            W[:, h, :, :, WOFF[i] + ppv, :] = blk.transpose(4, 2, 1, 3, 0)
    wpacks = [np.ascontiguousarray(W[m].reshape(2 * C, WCOLS))
              for m in range(NCORES)]

    # strip pack: S[m, c, g*SBANKC + s*64 + o] = w[o, c, 128m + 8g + A0[3+s], 6]
    # (the k=6 contribution of even bank positions 0,2,4,6; h=0 column only)
    Sp = np.empty((NCORES, C, NBANK, 4, O), f16)
    pg = m_ + g_ + np.array([A0[3 + s] for s in range(4)])[None, None, :]
    blk = wh[:, :, pg, KW - 1]                                # (O, C, M, G, 4)
    Sp[:] = blk.transpose(2, 1, 3, 4, 0)
    spacks = [np.ascontiguousarray(Sp[m].reshape(C, SCOLS))
              for m in range(NCORES)]

    # bias pack: [1, PC*O + B]; after the bias come B ones (stationary
    # operand of the bias matmul that opens each bank's accumulation group).
    bt = np.ascontiguousarray(bh.T)                           # (L, O)
    bpacks = []
    for m in range(NCORES):
        bp = np.empty((1, PC * O + B), f16)
        bp[0, :PC * O] = bt[PC * m: PC * m + PC].reshape(-1)
        bp[0, PC * O:] = 1.0
        bpacks.append(bp)
    return xpacks, wpacks, spacks, bpacks


_PROG = None


def _build_program():
    global _PROG
    if _PROG is not None:
        return _PROG

    import concourse.bacc as bacc
    import concourse.mybir as mybir
    import concourse.tile as tile

    F16 = mybir.dt.float16
    F32 = mybir.dt.float32

    nc = bacc.Bacc("TRN2", target_bir_lowering=False, debug=False,
                   num_devices=NCORES)
    x_d = nc.dram_tensor("xp", (2 * C, NT, B), F16, kind="ExternalInput")
    w_d = nc.dram_tensor("wp", (2 * C, WCOLS), F16, kind="ExternalInput")
    s_d = nc.dram_tensor("sp", (C, SCOLS), F16, kind="ExternalInput")
    b_d = nc.dram_tensor("bp", (1, PC * O + B), F16, kind="ExternalInput")
    o_d = nc.dram_tensor("out", (B, PC * O), F16, kind="ExternalOutput")

    with tile.TileContext(nc) as tc:
        with (
            tc.tile_pool(name="xb", bufs=2) as xpool,
            tc.tile_pool(name="wb", bufs=len(WCHUNKS)) as wpool,
            tc.tile_pool(name="sb", bufs=2) as stpool,
            tc.tile_pool(name="cst", bufs=1) as cpool,
            tc.tile_pool(name="st", bufs=4) as spool,
            tc.tile_pool(name="ps", bufs=4, space="PSUM") as ppool,
        ):
            # input DMA issue order: x0, strips, w0, bias, w1..w3, x1, w4..
            # — the sync HWDGE ring drains FIFO, so banks 0-7 can start
            # after x0+w0 while the rest of the weight streams behind them.
            # The tiny bias DMA goes after w0 so its issue time doesn't
            # delay the stream-critical transfers.
            xt0 = xpool.tile([2 * C, XSPLIT * B], F16)
            nc.sync.dma_start(xt0[:], x_d[:, :XSPLIT, :])
            st0 = stpool.tile([C, SCOLS], F16)
            nc.sync.dma_start(st0[:], s_d[:])
            biast = cpool.tile([1, PC * O + B], F16)
            wts = []
            for q, ch in enumerate(WCHUNKS):
                if q == 1:
                    nc.sync.dma_start(biast[:], b_d[:])
                if q == 4:
                    xt1 = xpool.tile([2 * C, (NT - XSPLIT) * B], F16)
                    nc.sync.dma_start(xt1[:], x_d[:, XSPLIT:, :])
                c0, c1 = _chunk_cols(ch)
                wt = wpool.tile([2 * C, c1 - c0], F16)
                nc.sync.dma_start(wt[:], w_d[:, c0:c1])
                wts.append((ch, c0, wt))
            ones = biast[0:1, PC * O: PC * O + B]

            def wslice(g, i):
                for (g0, i0, nb), c0, wt in wts:
                    b0 = 7 * g0 + i0
                    if b0 <= 7 * g + i < b0 + nb:
                        a = g * BANKC + WOFF[i] * O - c0
                        return wt[:, a: a + WN[i] * O]
                raise AssertionError

            def xslice(t, rows=2 * C):
                if t < XSPLIT:
                    return xt0[:rows, B * t: B * (t + 1)]
                return xt1[:rows, B * (t - XSPLIT): B * (t - XSPLIT + 1)]

            stage = None
            for g in range(NBANK):
                # last two banks get their own stage + store so the final
                # eviction->store chain after the last matmul is short
                solo = g >= NBANK - 2
                if solo:
                    stage = spool.tile([B, BANKW], F16)
                elif g % 2 == 0:
                    stage = spool.tile([B, 2 * BANKW], F16)
                ps = ppool.tile([B, BANKW], F32, tag="ps")
                # bias opens the accumulation group writing the full bank so
                # every accumulating piece lands on written psum.
                nc.tensor.matmul(
                    ps[:],
                    ones,
                    biast[0:1, BANKW * g: BANKW * (g + 1)],
                    start=True, stop=False,
                )
                # dense 64-row strips: k=6 of even bank positions 0,2,4,6
                # (base partition 0, same as the 128-row matmuls below)
                stt = st0
                sc0 = g * SBANKC
                for s in range(4):
                    t = 4 * g + 3 + s
                    nc.tensor.matmul(
                        ps[:, A0[3 + s] * O: (A0[3 + s] + 1) * O],
                        xslice(t, rows=C),
                        stt[:, sc0 + s * O: sc0 + (s + 1) * O],
                        start=False, stop=False,
                    )
                for i in range(7):
                    a = A0[i] + S_I[i]
                    nc.tensor.matmul(
                        ps[:, a * O: (a + WN[i]) * O],
                        xslice(4 * g + i),
                        wslice(g, i),
                        start=False,
                        stop=(i == 6),
                    )
                if solo:
                    # split the cast across DVE and ACT so the tail is short;
                    # solo stores go on the scalar HWDGE ring (their sem-lane
                    # predecessors are early input DMAs, long complete)
                    half = BANKW // 2
                    nc.vector.tensor_copy(stage[:, :half], ps[:, :half])
                    nc.scalar.copy(stage[:, half:], ps[:, half:])
                    nc.scalar.dma_start(
                        o_d[:, BANKW * g: BANKW * (g + 1)], stage[:])
                else:
                    sl = stage[:, BANKW * (g % 2): BANKW * (g % 2 + 1)]
                    if g % 2 == 0:
                        nc.vector.tensor_copy(sl, ps[:])
                    else:
                        nc.scalar.copy(sl, ps[:])
                    if g % 2 == 1:
                        # mid-stream stores use SWDGE (gpsimd) so they take
                        # DMASW sem lanes and never stall the HWDGE input
                        # rotation behind compute-gated completions
                        gb = g // 2
                        nc.gpsimd.dma_start(
                            o_d[:, 2 * BANKW * gb: 2 * BANKW * (gb + 1)],
                            stage[:])

    nc.compile()
    _PROG = nc
    return nc


def _ensure_ntff_hook():
    """bass_utils' trace path imports antenv.axon_hooks, which this image
    lacks — if BASS_TRACE is set in the environment that import would crash.
    Install a minimal shim (ctypes into libaxon_pjrt.so; falls back to a
    no-hook stub that bass_utils handles by skipping the trace)."""
    import sys
    import types
    try:
        import antenv.axon_hooks  # noqa: F401
        return
    except ImportError:
        pass
    hook = None
    try:
        import contextlib
        import ctypes
        lib = ctypes.CDLL("/opt/axon/libaxon_pjrt.so")
        lib.axon_start_nrt_profile.argtypes = [
            ctypes.POINTER(ctypes.c_int64), ctypes.c_size_t]
        lib.axon_start_nrt_profile.restype = ctypes.c_int64
        lib.axon_stop_nrt_profile.argtypes = [ctypes.c_char_p]
        lib.axon_stop_nrt_profile.restype = ctypes.c_int64

        @contextlib.contextmanager
        def _hook(output_dir, device_ids):
            import jax
            jax.devices()
            if device_ids:
                ids = (ctypes.c_int64 * len(device_ids))(*device_ids)
                rc = lib.axon_start_nrt_profile(ids, len(device_ids))
            else:
                rc = lib.axon_start_nrt_profile(None, 0)
            if rc != 0:
                raise RuntimeError(f"axon_start_nrt_profile rc={rc}")
            try:
                yield
            finally:
                lib.axon_stop_nrt_profile(str(output_dir).encode())

        hook = _hook
    except Exception:
        hook = None
    mod = types.ModuleType("antenv.axon_hooks")
    mod.get_axon_ntff_profile_hook = lambda: hook
    mod.set_axon_ntff_profile_hook = lambda h: None
    try:
        import antenv
        antenv.axon_hooks = mod
    except ImportError:
        pass
    sys.modules["antenv.axon_hooks"] = mod


def _run(x, weight, bias, trace=False, tmpdir=None):
    from concourse.bass_utils import run_bass_kernel_spmd
    _ensure_ntff_hook()

    xpacks, wpacks, spacks, bpacks = _pack_inputs(x, weight, bias)
    nc = _build_program()
    in_maps = [{"xp": xpacks[m], "wp": wpacks[m], "sp": spacks[m],
                "bp": bpacks[m]} for m in range(NCORES)]
    res = run_bass_kernel_spmd(nc, in_maps, list(range(NCORES)), trace=trace,
                               tmpdir=tmpdir)
    outs = [np.asarray(r["out"], np.float32).reshape(B, PC, O).transpose(0, 2, 1)
            for r in res.results]
    full = np.ascontiguousarray(np.concatenate(outs, axis=2))
    return full, res


def kernel(x, weight, bias):
    out, _ = _run(x, weight, bias, trace=False)
    return out


# revision 11
# speedup vs baseline: 1.0333x; 1.0333x over previous
"""LocallyConnected1d (untied-weight conv1d) on 8 Trainium2 NeuronCores.

Problem (hardcoded):
    x:      (B=128, C=64, L=1024) f32
    weight: (O=64, C=64, P=1024, K=7) f32   (untied per output position)
    bias:   (O=64, P=1024) f32
    out:    (B=128, O=64, P=1024) f32
    out[b,o,p] = sum_{c,k} xpad[b,c,p+k] * w[o,c,p,k] + bias[o,p]  (pad=3)

Sharding: sequence-parallel over P — core m owns positions [128m, 128m+128).
Each weight element is used exactly once, so this minimizes HBM traffic.

The kernel is HBM-DMA-bound, so everything streams as fp16 (x, weight,
bias, output): per-core traffic is ~12.2 MB vs 23.4 MB for f32, and fp16
matmul runs at full PE rate. Accumulation stays in f32 PSUM; fp16
quantization of inputs+output keeps rel-err ~1e-3.

Per-core algorithm: adjacent input columns (2t, 2t+1) are stacked into
128 contraction rows (2 cols x 64 ch), so each matmul uses the full PE
height. The stationary operand is the x column-pair [(h,c)=128, b=128];
the moving operand is that pair's untied weights [(h,c)=128, n*64] for
the n output positions the pair feeds inside the current PSUM bank of 8
positions (bank = [b=128, (pr,o)=512] f32 = one 2KB bank). A bank needs
column pairs 4g..4g+6.

A pair covering a position at the edge of its k-window only uses one of
its two columns; the other column's 64 rows would be zeros. For the
four k=7-side edges per bank (blocks i=3..6, even bank positions
0/2/4/6, always the h=0 column with k=6) the zero half is not stored:
those positions get a dense 64-row base-0 matmul from a separate strip
pack instead. The k=-1-side edges (h=1-only, which would need base-64
matmuls that cannot share a PSUM bank with base-0 ones) stay zero-padded
in the main pack. Bias opens each bank's accumulation group via a K=1
ones-x-bias matmul; eviction downcasts PSUM f32 -> fp16 on DVE/ACT.
"""

import numpy as np

B = 128
C = 64
O = 64
L = 1024
KW = 7
PAD = 3
NCORES = 8
PC = L // NCORES          # positions per core = 128
NJ = PC + 2 * PAD         # input columns per core incl halo = 134
NT = NJ // 2              # column pairs = 67
NBANK = PC // 8           # psum banks of 8 positions = 16
BANKW = 8 * O             # psum bank free width = 512 f32
XSPLIT = 36               # x pairs in the first DMA chunk (banks 0-7 use <=34)

# pair i = 4g+i covers bank-g positions [A0[i], A0[i]+N_I[i]) (bank-local)
N_I = [2, 4, 6, 8, 6, 4, 2]
A0 = [0, 0, 0, 0, 2, 4, 6]
# main-pack blocks: for i>=3 the first position (h=0-only, k=6) moves to the
# strip pack, so the stored block covers positions [A0[i]+S_I[i], ...).
S_I = [0, 0, 0, 1, 1, 1, 1]
WN = [N_I[i] - S_I[i] for i in range(7)]          # [2,4,6,7,5,3,1]
WOFF = np.cumsum([0] + WN).tolist()               # [0,2,6,12,19,24,27,28]
BANKC = WOFF[-1] * O                              # main cols per bank = 1792
WCOLS = NBANK * BANKC                             # main cols per core = 28672
SBANKC = 4 * O                                    # strip cols per bank = 256
SCOLS = NBANK * SBANKC                            # strip cols per core = 4096

# weight DMA chunks, as (first bank, first block, n blocks) — banks 0..13 in
# 2-bank chunks, then bank 14, then bank 15 split in two so the final
# compute chases the last bytes of the stream. Kept >=10 chunks of ~1MB so
# the ~8-deep HWDGE in-flight window always holds several MB of stream.
WCHUNKS = [(2 * q, 0, 14) for q in range(7)] + \
          [(14, 0, 7), (15, 0, 4), (15, 4, 3)]


def _chunk_cols(ch):
    g0, i0, nb = ch
    c0 = g0 * BANKC + WOFF[i0] * O
    i1 = i0 + nb
    c1 = (g0 + (i1 // 7)) * BANKC + WOFF[i1 % 7] * O
    return c0, c1


def _pack_inputs(x, weight, bias):
    """Host-side relayout into DMA/matmul-friendly fp16 per-core arrays."""
    f16 = np.float16
    xh = np.asarray(x, np.float32).astype(f16)
    wh = np.asarray(weight, np.float32).astype(f16)
    bh = np.asarray(bias, np.float32).astype(f16)

    xp = np.zeros((B, C, L + 2 * PAD), f16)
    xp[:, :, PAD:PAD + L] = xh
    xt = np.ascontiguousarray(xp.transpose(1, 2, 0))      # (C, 1030, B)
    xpacks = []
    for m in range(NCORES):
        s = xt[:, PC * m: PC * m + NJ, :]                 # (C, NJ, B)
        s = s.reshape(C, NT, 2, B).transpose(2, 0, 1, 3)  # (h, C, NT, B)
        xpacks.append(np.ascontiguousarray(s.reshape(2 * C, NT, B)))

    # main weight pack: W[m, h*C+c, g*BANKC + (WOFF[i]+pp')*64 + o]
    #   = w[o, c, 128m + 8g + A0[i] + S_I[i] + pp', k],
    #     k = 2i + h - A0[i] - S_I[i] - pp'   (zero where k outside [0,7))
    W = np.zeros((NCORES, 2, C, NBANK, WOFF[-1], O), f16)
    m_ = 128 * np.arange(NCORES)[:, None, None]
    g_ = 8 * np.arange(NBANK)[None, :, None]
    for i in range(7):
        for h in range(2):
            pp = np.arange(WN[i])
            kk = 2 * i + h - A0[i] - S_I[i] - pp
            sel = (kk >= 0) & (kk < KW)
            ppv, kkv = pp[sel], kk[sel]
            if ppv.size == 0:
                continue
            pg = m_ + g_ + A0[i] + S_I[i] + ppv[None, None, :]  # (M, G, nv)
            blk = wh[:, :, pg, kkv[None, None, :]]              # (O, C, M, G, nv)
            W[:, h, :, :, WOFF[i] + ppv, :] = blk.transpose(4, 2, 1, 3, 0)
    wpacks = [np.ascontiguousarray(W[m].reshape(2 * C, WCOLS))
              for m in range(NCORES)]

    # strip pack: S[m, c, g*SBANKC + s*64 + o] = w[o, c, 128m + 8g + A0[3+s], 6]
    # (the k=6 contribution of even bank positions 0,2,4,6; h=0 column only)
    Sp = np.empty((NCORES, C, NBANK, 4, O), f16)
    pg = m_ + g_ + np.array([A0[3 + s] for s in range(4)])[None, None, :]
    blk = wh[:, :, pg, KW - 1]                                # (O, C, M, G, 4)
    Sp[:] = blk.transpose(2, 1, 3, 4, 0)
    spacks = [np.ascontiguousarray(Sp[m].reshape(C, SCOLS))
              for m in range(NCORES)]

    # bias pack: [1, PC*O + B]; after the bias come B ones (stationary
    # operand of the bias matmul that opens each bank's accumulation group).
    bt = np.ascontiguousarray(bh.T)                           # (L, O)
    bpacks = []
    for m in range(NCORES):
        bp = np.empty((1, PC * O + B), f16)
        bp[0, :PC * O] = bt[PC * m: PC * m + PC].reshape(-1)
        bp[0, PC * O:] = 1.0
        bpacks.append(bp)
    return xpacks, wpacks, spacks, bpacks


_PROG = None


def _build_program():
    global _PROG
    if _PROG is not None:
        return _PROG

    import concourse.bacc as bacc
    import concourse.mybir as mybir
    import concourse.tile as tile

    F16 = mybir.dt.float16
    F32 = mybir.dt.float32

    nc = bacc.Bacc("TRN2", target_bir_lowering=False, debug=False,
                   num_devices=NCORES)
    x_d = nc.dram_tensor("xp", (2 * C, NT, B), F16, kind="ExternalInput")
    w_d = nc.dram_tensor("wp", (2 * C, WCOLS), F16, kind="ExternalInput")
    s_d = nc.dram_tensor("sp", (C, SCOLS), F16, kind="ExternalInput")
    b_d = nc.dram_tensor("bp", (1, PC * O + B), F16, kind="ExternalInput")
    o_d = nc.dram_tensor("out", (B, PC * O), F16, kind="ExternalOutput")

    with tile.TileContext(nc) as tc:
        with (
            tc.tile_pool(name="xb", bufs=2) as xpool,
            tc.tile_pool(name="wb", bufs=len(WCHUNKS)) as wpool,
            tc.tile_pool(name="sb", bufs=2) as stpool,
            tc.tile_pool(name="cst", bufs=1) as cpool,
            tc.tile_pool(name="st", bufs=4) as spool,
            tc.tile_pool(name="ps", bufs=4, space="PSUM") as ppool,
        ):
            # input DMA issue order: bias, x0, strips, w0..w3, x1, w4.. —
            # the sync HWDGE ring drains FIFO, so banks 0-7 can start after
            # x0+w0 while the rest of the weight streams behind them. Bias
            # is tiny but must land first: it gates every bank's opener
            # matmul (the start=True of the accumulation group).
            biast = cpool.tile([1, PC * O + B], F16)
            nc.sync.dma_start(biast[:], b_d[:])
            ones = biast[0:1, PC * O: PC * O + B]
            xt0 = xpool.tile([2 * C, XSPLIT * B], F16)
            nc.sync.dma_start(xt0[:], x_d[:, :XSPLIT, :])
            st0 = stpool.tile([C, SCOLS], F16)
            nc.sync.dma_start(st0[:], s_d[:])
            wts = []
            for q, ch in enumerate(WCHUNKS):
                if q == 4:
                    xt1 = xpool.tile([2 * C, (NT - XSPLIT) * B], F16)
                    nc.sync.dma_start(xt1[:], x_d[:, XSPLIT:, :])
                c0, c1 = _chunk_cols(ch)
                wt = wpool.tile([2 * C, c1 - c0], F16)
                nc.sync.dma_start(wt[:], w_d[:, c0:c1])
                wts.append((ch, c0, wt))

            def wslice(g, i):
                for (g0, i0, nb), c0, wt in wts:
                    b0 = 7 * g0 + i0
                    if b0 <= 7 * g + i < b0 + nb:
                        a = g * BANKC + WOFF[i] * O - c0
                        return wt[:, a: a + WN[i] * O]
                raise AssertionError

            def xslice(t, rows=2 * C):
                if t < XSPLIT:
                    return xt0[:rows, B * t: B * (t + 1)]
                return xt1[:rows, B * (t - XSPLIT): B * (t - XSPLIT + 1)]

            stage = None
            for g in range(NBANK):
                # last two banks get their own stage + store so the final
                # eviction->store chain after the last matmul is short
                solo = g >= NBANK - 2
                if solo:
                    stage = spool.tile([B, BANKW], F16)
                elif g % 2 == 0:
                    stage = spool.tile([B, 2 * BANKW], F16)
                ps = ppool.tile([B, BANKW], F32, tag="ps")
                # bias opens the accumulation group writing the full bank so
                # every accumulating piece lands on written psum.
                nc.tensor.matmul(
                    ps[:],
                    ones,
                    biast[0:1, BANKW * g: BANKW * (g + 1)],
                    start=True, stop=False,
                )
                # dense 64-row strips: k=6 of even bank positions 0,2,4,6
                # (base partition 0, same as the 128-row matmuls below)
                stt = st0
                sc0 = g * SBANKC
                for s in range(4):
                    t = 4 * g + 3 + s
                    nc.tensor.matmul(
                        ps[:, A0[3 + s] * O: (A0[3 + s] + 1) * O],
                        xslice(t, rows=C),
                        stt[:, sc0 + s * O: sc0 + (s + 1) * O],
                        start=False, stop=False,
                    )
                for i in range(7):
                    a = A0[i] + S_I[i]
                    nc.tensor.matmul(
                        ps[:, a * O: (a + WN[i]) * O],
                        xslice(4 * g + i),
                        wslice(g, i),
                        start=False,
                        stop=(i == 6),
                    )
                if solo:
                    # split the cast across DVE and ACT so the tail is short;
                    # solo stores go on the scalar HWDGE ring (their sem-lane
                    # predecessors are early input DMAs, long complete)
                    half = BANKW // 2
                    nc.vector.tensor_copy(stage[:, :half], ps[:, :half])
                    nc.scalar.copy(stage[:, half:], ps[:, half:])
                    nc.scalar.dma_start(
                        o_d[:, BANKW * g: BANKW * (g + 1)], stage[:])
                else:
                    sl = stage[:, BANKW * (g % 2): BANKW * (g % 2 + 1)]
                    if g % 2 == 0:
                        nc.vector.tensor_copy(sl, ps[:])
                    else:
                        nc.scalar.copy(sl, ps[:])
                    if g % 2 == 1:
                        # mid-stream stores use SWDGE (gpsimd) so they take
                        # DMASW sem lanes and never stall the HWDGE input
                        # rotation behind compute-gated completions
                        gb = g // 2
                        nc.gpsimd.dma_start(
                            o_d[:, 2 * BANKW * gb: 2 * BANKW * (gb + 1)],
                            stage[:])

    nc.compile()
    _PROG = nc
    return nc


def _ensure_ntff_hook():
    """bass_utils' trace path imports antenv.axon_hooks, which this image
    lacks — if BASS_TRACE is set in the environment that import would crash.
    Install a minimal shim (ctypes into libaxon_pjrt.so; falls back to a
    no-hook stub that bass_utils handles by skipping the trace)."""
    import sys
    import types
    try:
        import antenv.axon_hooks  # noqa: F401
        return
    except ImportError:
        pass
    hook = None
    try:
        import contextlib
        import ctypes
        lib = ctypes.CDLL("/opt/axon/libaxon_pjrt.so")
        lib.axon_start_nrt_profile.argtypes = [
            ctypes.POINTER(ctypes.c_int64), ctypes.c_size_t]
        lib.axon_start_nrt_profile.restype = ctypes.c_int64
        lib.axon_stop_nrt_profile.argtypes = [ctypes.c_char_p]
        lib.axon_stop_nrt_profile.restype = ctypes.c_int64

        @contextlib.contextmanager
        def _hook(output_dir, device_ids):
            import jax
            jax.devices()
            if device_ids:
                ids = (ctypes.c_int64 * len(device_ids))(*device_ids)
                rc = lib.axon_start_nrt_profile(ids, len(device_ids))
            else:
                rc = lib.axon_start_nrt_profile(None, 0)
            if rc != 0:
                raise RuntimeError(f"axon_start_nrt_profile rc={rc}")
            try:
                yield
            finally:
                lib.axon_stop_nrt_profile(str(output_dir).encode())

        hook = _hook
    except Exception:
        hook = None
    mod = types.ModuleType("antenv.axon_hooks")
    mod.get_axon_ntff_profile_hook = lambda: hook
    mod.set_axon_ntff_profile_hook = lambda h: None
    try:
        import antenv
        antenv.axon_hooks = mod
    except ImportError:
        pass
    sys.modules["antenv.axon_hooks"] = mod


def _run(x, weight, bias, trace=False, tmpdir=None):
    from concourse.bass_utils import run_bass_kernel_spmd
    _ensure_ntff_hook()

    xpacks, wpacks, spacks, bpacks = _pack_inputs(x, weight, bias)
    nc = _build_program()
    in_maps = [{"xp": xpacks[m], "wp": wpacks[m], "sp": spacks[m],
                "bp": bpacks[m]} for m in range(NCORES)]
    res = run_bass_kernel_spmd(nc, in_maps, list(range(NCORES)), trace=trace,
                               tmpdir=tmpdir)
    outs = [np.asarray(r["out"], np.float32).reshape(B, PC, O).transpose(0, 2, 1)
            for r in res.results]
    full = np.ascontiguousarray(np.concatenate(outs, axis=2))
    return full, res


def kernel(x, weight, bias):
    out, _ = _run(x, weight, bias, trace=False)
    return out


# revision 16
# speedup vs baseline: 1.1624x; 1.1249x over previous
"""LocallyConnected1d (untied-weight conv1d) on 8 Trainium2 NeuronCores.

Problem (hardcoded):
    x:      (B=128, C=64, L=1024) f32
    weight: (O=64, C=64, P=1024, K=7) f32   (untied per output position)
    bias:   (O=64, P=1024) f32
    out:    (B=128, O=64, P=1024) f32
    out[b,o,p] = sum_{c,k} xpad[b,c,p+k] * w[o,c,p,k] + bias[o,p]  (pad=3)

Sharding: sequence-parallel over P — core m owns positions [128m, 128m+128).
Each weight element is used exactly once, so this minimizes HBM traffic.

The kernel is HBM-DMA-bound, so everything streams as fp16 (x, weight,
bias, output): ~12.7 MB/core vs 23.4 MB for f32, and fp16 matmul runs at
full PE rate. Accumulation stays in f32 PSUM; fp16 quantization of
inputs+output keeps rel-err ~5e-4 (gate is 2e-2).

Per-core algorithm: adjacent input columns (2t, 2t+1) are stacked into
128 contraction rows (2 cols x 64 ch) so each matmul uses the full PE
height. Stationary operand = the x column-pair [(h,c)=128, b=128];
moving operand = that pair's untied weights [(h,c)=128, n*64] for the n
output positions the pair feeds inside the current PSUM bank of 8
positions (one 2KB bank = [b=128, (pr,o)=512] f32). A bank needs column
pairs 4g..4g+6 with coverage n = 2,4,6,8,6,4,2; (p,k) combos outside the
window are zero-padded in the weight pack. Bias opens each bank's
accumulation group via a K=1 ones-x-bias matmul (so the bias DMA must
land first). Eviction downcasts PSUM f32 -> fp16 (DVE; the final two
banks split DVE/ACT for a short tail).

Pipeline notes (probed on HW): HWDGE DMA issue is throttled ~8-deep via
8 round-robin completion-sem lanes shared by ALL HWDGE DMAs in scheduled
order, so mid-stream output stores go on SWDGE (gpsimd) to keep them out
of the input rotation; 64-partition side DMAs (tried for a padding-free
weight pack) drain on half the SDMA engines and skew chunk completions —
avoided. The last weight chunks shrink (bank 14, then bank 15 in two
pieces) so the final matmuls chase the stream instead of trailing it.
"""

import numpy as np

B = 128
C = 64
O = 64
L = 1024
KW = 7
PAD = 3
NCORES = 8
PC = L // NCORES          # positions per core = 128
NJ = PC + 2 * PAD         # input columns per core incl halo = 134
NT = NJ // 2              # column pairs = 67
NBANK = PC // 8           # psum banks of 8 positions = 16
BANKW = 8 * O             # psum bank free width = 512 f32
XSPLIT = 36               # x pairs in the first DMA chunk (banks 0-7 use <=34)

N_I = [2, 4, 6, 8, 6, 4, 2]
A0 = [0, 0, 0, 0, 2, 4, 6]
CUM = [0, 2, 6, 12, 20, 26, 30]
BANKC = 32 * O
WCOLS = NBANK * BANKC

# weight DMA chunks as column ranges: banks 0..13 in 2-bank (1MB) chunks,
# then bank 14, then bank 15 split at block 4 — the tail chunks shrink so
# the final compute chases the last bytes of the stream.
WCHUNKS = [(2 * q * BANKC, (2 * q + 2) * BANKC) for q in range(7)]
WCHUNKS += [(14 * BANKC, 15 * BANKC),
            (15 * BANKC, 15 * BANKC + CUM[4] * O),
            (15 * BANKC + CUM[4] * O, 16 * BANKC)]


def _wchunk_of(g, i):
    """Index of the chunk holding bank g's block i."""
    if g < 14:
        return g // 2
    if g == 14:
        return 7
    return 8 if i < 4 else 9


def _pack_inputs(x, weight, bias):
    f16 = np.float16
    xh = np.asarray(x, np.float32).astype(f16)
    wh = np.asarray(weight, np.float32).astype(f16)
    bh = np.asarray(bias, np.float32).astype(f16)

    xp = np.zeros((B, C, L + 2 * PAD), f16)
    xp[:, :, PAD:PAD + L] = xh
    xt = np.ascontiguousarray(xp.transpose(1, 2, 0))
    xpacks = []
    for m in range(NCORES):
        s = xt[:, PC * m: PC * m + NJ, :]
        s = s.reshape(C, NT, 2, B).transpose(2, 0, 1, 3)
        xpacks.append(np.ascontiguousarray(s.reshape(2 * C, NT, B)))

    W = np.zeros((NCORES, 2, C, NBANK, 32, O), f16)
    m_ = 128 * np.arange(NCORES)[:, None, None]
    g_ = 8 * np.arange(NBANK)[None, :, None]
    for i in range(7):
        for h in range(2):
            pp = np.arange(N_I[i])
            kk = 2 * i + h - A0[i] - pp
            sel = (kk >= 0) & (kk < KW)
            ppv, kkv = pp[sel], kk[sel]
            if ppv.size == 0:
                continue
            pg = m_ + g_ + A0[i] + ppv[None, None, :]
            blk = wh[:, :, pg, kkv[None, None, :]]
            W[:, h, :, :, CUM[i] + ppv, :] = blk.transpose(4, 2, 1, 3, 0)
    wpacks = [np.ascontiguousarray(W[m].reshape(2 * C, WCOLS))
              for m in range(NCORES)]

    bt = np.ascontiguousarray(bh.T)
    bpacks = []
    for m in range(NCORES):
        bp = np.empty((1, PC * O + B), f16)
        bp[0, :PC * O] = bt[PC * m: PC * m + PC].reshape(-1)
        bp[0, PC * O:] = 1.0
        bpacks.append(bp)
    return xpacks, wpacks, bpacks


_PROG = None


def _build_program():
    global _PROG
    if _PROG is not None:
        return _PROG

    import concourse.bacc as bacc
    import concourse.mybir as mybir
    import concourse.tile as tile

    F16 = mybir.dt.float16
    F32 = mybir.dt.float32

    nc = bacc.Bacc("TRN2", target_bir_lowering=False, debug=False,
                   num_devices=NCORES)
    x_d = nc.dram_tensor("xp", (2 * C, NT, B), F16, kind="ExternalInput")
    w_d = nc.dram_tensor("wp", (2 * C, WCOLS), F16, kind="ExternalInput")
    b_d = nc.dram_tensor("bp", (1, PC * O + B), F16, kind="ExternalInput")
    o_d = nc.dram_tensor("out", (B, PC * O), F16, kind="ExternalOutput")

    with tile.TileContext(nc) as tc:
        with (
            tc.tile_pool(name="xb", bufs=2) as xpool,
            tc.tile_pool(name="wb", bufs=8) as wpool,
            tc.tile_pool(name="cst", bufs=1) as cpool,
            tc.tile_pool(name="st", bufs=4) as spool,
            tc.tile_pool(name="ps", bufs=4, space="PSUM") as ppool,
        ):
            biast = cpool.tile([1, PC * O + B], F16)
            nc.sync.dma_start(biast[:], b_d[:])
            ones = biast[0:1, PC * O: PC * O + B]

            xt0 = xpool.tile([2 * C, XSPLIT * B], F16)
            nc.sync.dma_start(xt0[:], x_d[:, :XSPLIT, :])
            wts = []
            for q, (c0, c1) in enumerate(WCHUNKS):
                if q == 4:
                    xt1 = xpool.tile([2 * C, (NT - XSPLIT) * B], F16)
                    nc.sync.dma_start(xt1[:], x_d[:, XSPLIT:, :])
                wt = wpool.tile([2 * C, c1 - c0], F16)
                nc.sync.dma_start(wt[:], w_d[:, c0:c1])
                wts.append((c0, wt))

            stage = None
            for g in range(NBANK):
                solo = g >= NBANK - 2
                if solo:
                    stage = spool.tile([B, BANKW], F16)
                elif g % 2 == 0:
                    stage = spool.tile([B, 2 * BANKW], F16)
                ps = ppool.tile([B, BANKW], F32, tag="ps")
                nc.tensor.matmul(
                    ps[:],
                    ones,
                    biast[0:1, BANKW * g: BANKW * (g + 1)],
                    start=True, stop=False,
                )
                for i in range(7):
                    t = 4 * g + i
                    if t < XSPLIT:
                        xs = xt0[:, B * t: B * (t + 1)]
                    else:
                        xs = xt1[:, B * (t - XSPLIT): B * (t - XSPLIT + 1)]
                    wc0, wt = wts[_wchunk_of(g, i)]
                    w0 = g * BANKC + CUM[i] * O - wc0
                    ws = wt[:, w0: w0 + N_I[i] * O]
                    nc.tensor.matmul(
                        ps[:, A0[i] * O: (A0[i] + N_I[i]) * O],
                        xs,
                        ws,
                        start=False,
                        stop=(i == 6),
                    )
                if solo:
                    # split the cast across DVE and ACT so the tail after
                    # the final matmul is short; solo stores stay on the
                    # scalar HWDGE ring (their sem-lane predecessors are
                    # early input DMAs, long complete by then)
                    half = BANKW // 2
                    nc.vector.tensor_copy(stage[:, :half], ps[:, :half])
                    nc.scalar.copy(stage[:, half:], ps[:, half:])
                    nc.scalar.dma_start(
                        o_d[:, BANKW * g: BANKW * (g + 1)], stage[:])
                else:
                    sl = stage[:, BANKW * (g % 2): BANKW * (g % 2 + 1)]
                    nc.vector.tensor_copy(sl, ps[:])
                    if g % 2 == 1:
                        # mid-stream stores use SWDGE (gpsimd) so they take
                        # DMASW sem lanes and never stall the HWDGE input
                        # rotation behind compute-gated completions
                        gb = g // 2
                        nc.gpsimd.dma_start(
                            o_d[:, 2 * BANKW * gb: 2 * BANKW * (gb + 1)],
                            stage[:])

    nc.compile()
    _PROG = nc
    return nc


def _ensure_ntff_hook():
    import sys
    import types
    try:
        import antenv.axon_hooks  # noqa: F401
        return
    except ImportError:
        pass
    hook = None
    try:
        import contextlib
        import ctypes
        lib = ctypes.CDLL("/opt/axon/libaxon_pjrt.so")
        lib.axon_start_nrt_profile.argtypes = [
            ctypes.POINTER(ctypes.c_int64), ctypes.c_size_t]
        lib.axon_start_nrt_profile.restype = ctypes.c_int64
        lib.axon_stop_nrt_profile.argtypes = [ctypes.c_char_p]
        lib.axon_stop_nrt_profile.restype = ctypes.c_int64

        @contextlib.contextmanager
        def _hook(output_dir, device_ids):
            import jax
            jax.devices()
            if device_ids:
                ids = (ctypes.c_int64 * len(device_ids))(*device_ids)
                rc = lib.axon_start_nrt_profile(ids, len(device_ids))
            else:
                rc = lib.axon_start_nrt_profile(None, 0)
            if rc != 0:
                raise RuntimeError(f"axon_start_nrt_profile rc={rc}")
            try:
                yield
            finally:
                lib.axon_stop_nrt_profile(str(output_dir).encode())

        hook = _hook
    except Exception:
        hook = None
    mod = types.ModuleType("antenv.axon_hooks")
    mod.get_axon_ntff_profile_hook = lambda: hook
    mod.set_axon_ntff_profile_hook = lambda h: None
    try:
        import antenv
        antenv.axon_hooks = mod
    except ImportError:
        pass
    sys.modules["antenv.axon_hooks"] = mod


def _run(x, weight, bias, trace=False, tmpdir=None):
    from concourse.bass_utils import run_bass_kernel_spmd
    _ensure_ntff_hook()

    xpacks, wpacks, bpacks = _pack_inputs(x, weight, bias)
    nc = _build_program()
    in_maps = [{"xp": xpacks[m], "wp": wpacks[m], "bp": bpacks[m]}
               for m in range(NCORES)]
    res = run_bass_kernel_spmd(nc, in_maps, list(range(NCORES)), trace=trace,
                               tmpdir=tmpdir)
    outs = [np.asarray(r["out"], np.float32).reshape(B, PC, O).transpose(0, 2, 1)
            for r in res.results]
    full = np.ascontiguousarray(np.concatenate(outs, axis=2))
    return full, res


def kernel(x, weight, bias):
    out, _ = _run(x, weight, bias, trace=False)
    return out
